# revision 9
# baseline (speedup 1.0000x reference)
"""Single-head attention (B=4, S=4096, D=128), f32 in/out, on 8 TRN2 NeuronCores.

Sharding: data-parallel over (batch, query-half): core c handles batch c//2,
query rows (c%2)*2048 .. +2048. Weights replicated. Per-core flash-style
attention:
  - host pre-transposes x so d is on partitions (pure layout, numpy)
  - QKV projections on PE (f32; Q,K emitted bf16 with 1/sqrt(128) folded
    into Q; V packed bf16 as [k_part, kt, d])
  - pass 1: bf16 Q@K^T scores in [q_part, k_free] 1024-chunks -> DVE row-max
  - pass 2: same matmuls -> ACT exp(psum - max) with fused per-partition bias,
    bf16 probs out; accum_out gives the softmax denominator for free
  - DMA-transpose probs (bf16 XBAR path) into a per-group [k_part, 512_q] tile
  - PV on PE as out^T[d, q] with N=512 moving operand (vs N=128 the straight
    way -- 4x fewer matmul+ldweights pairs)
  - denominators: r=1/l per q-tile is DMA-reshaped into a [1, q] row, then a
    broadcast DMA makes [128, q] for the final DVE multiply; output is written
    transposed [d, q] and the host transposes back.

bf16 scores are safe here: measured rel_err vs f32 reference ~3e-3
(softmax is very peaked, but top-2 gaps are >1 for 95% of rows and bf16
score error is ~0.3 absolute).
"""

import math
from contextlib import ExitStack

import numpy as np

import concourse.bass as bass
import concourse.tile as tile
from concourse import bacc, mybir
from concourse.bass_utils import run_bass_kernel_spmd

P = 128
D = 128
B = 4
S = 4096
N_CORES = 8
SQ = S * B // N_CORES  # 2048 query rows per core
SK = S  # keys per core
NQT = SQ // P  # 16 query tiles
NKT = SK // P  # 32 key tiles
KC = 1024  # score chunk width (two PSUM banks)
NKC = SK // KC  # 4 chunks per query tile
QG = 512  # query group (4 q-tiles) for the PV matmul
NQG = SQ // QG
SCALE = 1.0 / math.sqrt(D)

F32 = mybir.dt.float32
BF16 = mybir.dt.bfloat16


def build_bass() -> bacc.Bacc:
    nc = bacc.Bacc("TRN2", target_bir_lowering=False, debug=False)

    xqT = nc.declare_dram_parameter("xqT", [P, SQ], F32, isOutput=False)
    xkT = nc.declare_dram_parameter("xkT", [P, SK], F32, isOutput=False)
    wq = nc.declare_dram_parameter("wq", [D, D], F32, isOutput=False)
    wk = nc.declare_dram_parameter("wk", [D, D], F32, isOutput=False)
    wv = nc.declare_dram_parameter("wv", [D, D], F32, isOutput=False)
    # output is [d, q]; host transposes back
    out_ext = nc.declare_dram_parameter("out", [D, SQ], F32, isOutput=True)

    with tile.TileContext(nc) as tc, ExitStack() as ctx:
        const = ctx.enter_context(tc.tile_pool(name="const", bufs=1))
        psAB = ctx.enter_context(tc.tile_pool(name="psAB", bufs=3, space="PSUM"))
        pspv = ctx.enter_context(tc.tile_pool(name="pspv", bufs=1, space="PSUM"))
        probs_pool = ctx.enter_context(tc.tile_pool(name="probs", bufs=3))
        pT_pool = ctx.enter_context(tc.tile_pool(name="probsT", bufs=2))
        stat = ctx.enter_context(tc.tile_pool(name="stat", bufs=4))
        rb_pool = ctx.enter_context(tc.tile_pool(name="rb", bufs=2))
        dram = ctx.enter_context(tc.tile_pool(name="rdram", bufs=1, space="DRAM"))
        out_pool = ctx.enter_context(tc.tile_pool(name="outp", bufs=2))

        # ---- load inputs ----
        xqT_sb = const.tile([P, SQ], F32)
        nc.sync.dma_start(xqT_sb[:], xqT[:])
        xkT_sb = const.tile([P, SK], F32)
        nc.sync.dma_start(xkT_sb[:], xkT[:])
        wq_sb = const.tile([D, D], F32)
        nc.sync.dma_start(wq_sb[:], wq[:])
        wk_sb = const.tile([D, D], F32)
        nc.sync.dma_start(wk_sb[:], wk[:])
        wv_sb = const.tile([D, D], F32)
        nc.sync.dma_start(wv_sb[:], wv[:])

        # rrow[0, q] collects 1/l per query (DMA-reshaped from [q_part, 1]);
        # lives in DRAM so the final [128, q] broadcast DMA is legal
        rrow = dram.tile([1, SQ], F32)

        # ---- projections ----
        # qbf[e, q] = sum_d wq[d, e] * xq[q, d] * SCALE   (bf16)
        qbf = const.tile([P, SQ], BF16)
        for i in range(SQ // KC):
            ps = psAB.tile([P, KC], F32, tag="ps")
            for h in range(2):
                nc.tensor.matmul(
                    ps[:, h * 512 : (h + 1) * 512],
                    lhsT=wq_sb[:],
                    rhs=xqT_sb[:, i * KC + h * 512 : i * KC + (h + 1) * 512],
                    start=True,
                    stop=True,
                )
            nc.scalar.activation(
                qbf[:, i * KC : (i + 1) * KC],
                ps[:],
                mybir.ActivationFunctionType.Copy,
                scale=SCALE,
            )
        kbf = const.tile([P, SK], BF16)
        for i in range(SK // KC):
            ps = psAB.tile([P, KC], F32, tag="ps")
            for h in range(2):
                nc.tensor.matmul(
                    ps[:, h * 512 : (h + 1) * 512],
                    lhsT=wk_sb[:],
                    rhs=xkT_sb[:, i * KC + h * 512 : i * KC + (h + 1) * 512],
                    start=True,
                    stop=True,
                )
            nc.scalar.activation(
                kbf[:, i * KC : (i + 1) * KC],
                ps[:],
                mybir.ActivationFunctionType.Copy,
            )
        # vbf[k_part, kt, d] = V[kt*128 + k_part, d]  (bf16), 8 k-tiles per copy
        vbf = const.tile([P, NKT, D], BF16)
        for t in range(NKT // 8):
            ps = psAB.tile([P, KC], F32, tag="ps")
            for j in range(8):
                kt = t * 8 + j
                nc.tensor.matmul(
                    ps[:, j * P : (j + 1) * P],
                    lhsT=xkT_sb[:, kt * P : (kt + 1) * P],
                    rhs=wv_sb[:],
                    start=True,
                    stop=True,
                )
            nc.scalar.activation(
                vbf[:, t * 8 : (t + 1) * 8, :].rearrange("p a b -> p (a b)"),
                ps[:],
                mybir.ActivationFunctionType.Copy,
            )

        # ---- attention ----
        def emit_pv(g, pTg_g):
            # PV for group g: poT[d, q] = sum_kt V-tile.T @ probsT-tile
            po = pspv.tile([P, QG], F32, tag="pv")
            for kt in range(NKT):
                nc.tensor.matmul(
                    po[:],
                    lhsT=vbf[:, kt, :],
                    rhs=pTg_g[:, kt, :],
                    start=(kt == 0),
                    stop=(kt == NKT - 1),
                )
            rb = rb_pool.tile([P, QG], F32, tag="rb")
            nc.gpsimd.dma_start(
                rb[:], rrow[0:1, g * QG : (g + 1) * QG].to_broadcast([P, QG])
            )
            ot = out_pool.tile([P, QG], F32, tag="ot")
            nc.vector.tensor_mul(ot[:], po[:], rb[:])
            nc.sync.dma_start(out_ext[:, g * QG : (g + 1) * QG], ot[:])

        pTg = None
        pending_pv = None  # (g, pTg) whose PV is deferred into the next group
        for qt in range(NQT):
            q_sl = qbf[:, qt * P : (qt + 1) * P]
            g = qt // 4
            gi = qt % 4

            # emit the previous group's PV two q-tiles into this group, so
            # its transposes drain behind this group's score matmuls
            if gi == 2 and pending_pv is not None:
                emit_pv(*pending_pv)
                pending_pv = None

            # pass 1: row maxes
            mx = stat.tile([P, NKC], F32, tag="mx")
            for c in range(NKC):
                ps = psAB.tile([P, KC], F32, tag="ps")
                for h in range(2):
                    nc.tensor.matmul(
                        ps[:, h * 512 : (h + 1) * 512],
                        lhsT=q_sl,
                        rhs=kbf[:, c * KC + h * 512 : c * KC + (h + 1) * 512],
                        start=True,
                        stop=True,
                    )
                nc.vector.reduce_max(
                    mx[:, c : c + 1], ps[:], axis=mybir.AxisListType.X
                )
            negm = stat.tile([P, 1], F32, tag="negm")
            nc.vector.tensor_reduce(
                negm[:], mx[:], axis=mybir.AxisListType.X,
                op=mybir.AluOpType.max, negate=True,
            )

            # pass 2: probs = exp(scores - max) bf16, accumulate row sums
            accs = stat.tile([P, NKC], F32, tag="accs")
            probs = probs_pool.tile([P, SK], BF16)
            for c in range(NKC):
                ps = psAB.tile([P, KC], F32, tag="ps")
                for h in range(2):
                    nc.tensor.matmul(
                        ps[:, h * 512 : (h + 1) * 512],
                        lhsT=q_sl,
                        rhs=kbf[:, c * KC + h * 512 : c * KC + (h + 1) * 512],
                        start=True,
                        stop=True,
                    )
                nc.scalar.activation(
                    probs[:, c * KC : (c + 1) * KC],
                    ps[:],
                    mybir.ActivationFunctionType.Exp,
                    bias=negm[:],
                    scale=1.0,
                    accum_out=accs[:, c : c + 1],
                )
            l_sum = stat.tile([P, 1], F32, tag="lsum")
            nc.vector.reduce_sum(l_sum[:], accs[:], axis=mybir.AxisListType.X)
            r_sb = stat.tile([P, 1], F32, tag="recip")
            nc.vector.reciprocal(r_sb[:], l_sum[:])
            # park 1/l as a row vector (q on the free axis)
            nc.gpsimd.dma_start(rrow[0:1, qt * P : (qt + 1) * P], r_sb[:])

            # transpose probs into the group tile [k_part, kt, q]
            if gi == 0:
                pTg = pT_pool.tile([P, NKT, QG], BF16)
            assert pTg is not None
            nc.sync.dma_start_transpose(
                pTg[:, :, gi * P : (gi + 1) * P], probs[:]
            )

            if gi == 3:
                pending_pv = (g, pTg)
        # last group's PV has nothing to hide behind; emit it now
        if pending_pv is not None:
            emit_pv(*pending_pv)

    nc.compile()
    return nc


_NC_CACHE: bacc.Bacc | None = None


def _get_nc() -> bacc.Bacc:
    global _NC_CACHE
    if _NC_CACHE is None:
        _NC_CACHE = build_bass()
    return _NC_CACHE


def kernel(**inputs: np.ndarray) -> np.ndarray:
    x = np.asarray(inputs["x"], dtype=np.float32)
    wq = np.ascontiguousarray(np.asarray(inputs["w_query"], dtype=np.float32))
    wk = np.ascontiguousarray(np.asarray(inputs["w_key"], dtype=np.float32))
    wv = np.ascontiguousarray(np.asarray(inputs["w_value"], dtype=np.float32))

    nc = _get_nc()

    in_maps = []
    for c in range(N_CORES):
        b = c // 2
        qoff = (c % 2) * SQ
        xT = np.ascontiguousarray(x[b].T)  # [128, 4096]
        xqT = np.ascontiguousarray(xT[:, qoff : qoff + SQ])  # [128, 2048]
        in_maps.append(
            {"xqT": xqT, "xkT": xT, "wq": wq, "wk": wk, "wv": wv}
        )

    res = run_bass_kernel_spmd(nc, in_maps, core_ids=list(range(N_CORES)))

    out = np.empty((B, S, D), dtype=np.float32)
    for c in range(N_CORES):
        b = c // 2
        qoff = (c % 2) * SQ
        out[b, qoff : qoff + SQ, :] = res.results[c]["out"].T
    return out
